# revision 6
# baseline (speedup 1.0000x reference)
"""ClassAttention (decode-style single-query attention) on 8 TRN2 NeuronCores.

Math (per batch b):
    kv = x @ Wkv              # [N, 2*H*D], k half cols 0:1024, v half 1024:2048
    q  = x[0] @ Wq            # [H*D]  (CLS token only)
    logits[t, h] = scale * sum_d q[h,d] * k[t, h*64+d]
    attn = softmax_t(logits)
    cls[h,d] = sum_t attn[t,h] * v[t, h*64+d]
    out = cls @ Wproj + bproj

v3 restructuring (v2 was PE-issue-bound: 3300 tiny matmuls, each paying a
full 128-col LDWEIGHTS for a 16-col moving operand):
  - All sweep matmuls are N=512 with TINY stationaries:
      logits^T[h, t] : lhsT = wkf block [128, 16], moving = x^T [128, 512]
      r[h, c]        : lhsT = e block   [128, 16], moving = x   [128, 512]
    so LDWEIGHTS is 16 cols (~15 ns) and each matmul streams 512 columns.
  - x^T comes from the DMA XBAR (SBUF->SBUF transposing DMA, one 2 MB call
    per 1024-row chunk) instead of 512 PE transposes.
  - Attention accumulation lives in PSUM chains (64 matmuls/batch), not
    vector-engine adds.
  - sum_t exp on the DVE (reduce over the free axis of e[16, 512]).
  - Weights stay f32 (HWDGE loads on sync/scalar queues) except Wv/Wproj
    which cast-load bf16 on the SWDGE queue between batch 0 and batch 1
    x streams; x cast-loads bf16 on SWDGE with 32 KB/partition descriptors.
  - Softmax still runs without max-subtraction (logits are O(1)) and the
    1/sum(exp) normalization is applied to the tiny r[h, c] tensor.

Sharding: pure data-parallel over B: 16 batches / 8 cores = 2 per core.
Weights are replicated; each core returns its [2, 1024] output shard.
"""

import numpy as np

import concourse.bass as bass
import concourse.mybir as mybir
import concourse.tile as tile
from concourse import bacc
from concourse.bass_utils import run_bass_kernel_spmd
from concourse.masks import make_identity

F32 = mybir.dt.float32
BF16 = mybir.dt.bfloat16

B, SEQ, C = 16, 4096, 1024
H, D = 16, 64
SCALE = D ** -0.5  # 0.125
N_CORES = 8
BPC = B // N_CORES          # batches per core
CB = C // 128               # 8 contraction blocks
RPC = 1024                  # seq rows per chunk
NCH = SEQ // RPC            # 4 chunks per batch
SUB = RPC // 128            # 8 sub-tiles (of 128 rows) per chunk


def _build():
    nc = bacc.Bacc(
        "TRN2", target_bir_lowering=False, debug=False, num_devices=N_CORES
    )
    x_ap = nc.dram_tensor("x", [BPC, SEQ, C], F32, kind="ExternalInput").ap()
    wq_ap = nc.dram_tensor("Wq", [C, H * D], F32, kind="ExternalInput").ap()
    wkv_ap = nc.dram_tensor("Wkv", [C, 2 * H * D], F32, kind="ExternalInput").ap()
    wp_ap = nc.dram_tensor("Wproj", [H * D, C], F32, kind="ExternalInput").ap()
    bp_ap = nc.dram_tensor("bproj", [C], F32, kind="ExternalInput").ap()
    out_ap = nc.dram_tensor("out", [BPC, C], F32, kind="ExternalOutput").ap()

    with tile.TileContext(nc) as tc:
        _emit(nc, tc, x_ap, wq_ap, wkv_ap, wp_ap, bp_ap, out_ap)
    nc.compile()
    return nc


def _emit(nc, tc, x_ap, wq_ap, wkv_ap, wp_ap, bp_ap, out_ap):
    with tc.tile_pool(name="consts", bufs=1) as consts:
        # Wv / Wproj: bf16 (finalize movers), SWDGE cast-loads QUEUED between
        # batch 0's and batch 1's x chunk loads (see below).
        wv_bf = consts.tile([128, CB, 1024], BF16)
        wp_bf = consts.tile([128, CB, 1024], BF16)

        bproj_sb = consts.tile([1, C], F32)
        nc.scalar.dma_start(bproj_sb[:], bp_ap[:].unsqueeze(0))

        # CLS rows of x, transposed on load: xcls[p, g, b] = x[b, 0, g*128+p]
        xcls_f = consts.tile([128, CB, BPC], F32)
        for b in range(BPC):
            nc.scalar.dma_start(
                xcls_f[:, :, b : b + 1],
                x_ap[b, 0:1, :].rearrange("o (g p) -> p g o", p=128),
            )

        sc_row = consts.tile([1, 128], F32)          # scale row: qb outer product
        nc.vector.memset(sc_row[:], SCALE)
        id16_bf = consts.tile([16, 16], BF16)        # PE-transpose identity (e)
        make_identity(nc, id16_bf[:])
        id16_f = consts.tile([16, 16], F32)          # PE-transpose identity (cls)
        make_identity(nc, id16_f[:])

        wkf_bf = [
            consts.tile([128, CB, H], BF16, tag=f"wkf{b}", name=f"wkf{b}")
            for b in range(BPC)
        ]
        qb_sb = [
            consts.tile([128, C], F32, tag=f"qb{b}", name=f"qb{b}")
            for b in range(BPC)
        ]

        with (
            tc.tile_pool(name="xbf", bufs=3) as xbf_pool,
            tc.tile_pool(name="xt", bufs=2) as xt_pool,
        ):
            # ---- SWDGE queue order: x b0 chunks | Wv, Wp | x b1 chunks ----
            x_tiles = {}

            def load_x_chunk(b, k):
                x_bf = xbf_pool.tile([128, SUB, 1024], BF16, tag="x")
                nc.gpsimd.dma_start(
                    x_bf[:],
                    x_ap[b, k * RPC : (k + 1) * RPC, :].rearrange(
                        "(p i) c -> p i c", p=128
                    ),
                )
                x_tiles[(b, k)] = x_bf

            for k in range(NCH):
                load_x_chunk(0, k)
            nc.gpsimd.dma_start(
                wv_bf[:], wkv_ap[:, 1024:2048].rearrange("(g p) n -> p g n", p=128)
            )
            nc.gpsimd.dma_start(
                wp_bf[:], wp_ap[:, :].rearrange("(g p) n -> p g n", p=128)
            )
            for k in range(NCH):
                load_x_chunk(1, k)

            # ---- qfold: Wq/Wk f32 in a scoped pool (freed after the fold) ----
            with (
                tc.tile_pool(name="wqk", bufs=1) as wqk,
                tc.tile_pool(name="fold", bufs=2) as fold_pool,
                tc.tile_pool(name="qps", bufs=2, space="PSUM") as qps,
                tc.tile_pool(name="qbps", bufs=1, space="PSUM") as qbps,
            ):
                wq_f = wqk.tile([128, CB, 1024], F32, tag="wq")
                wk_f = wqk.tile([128, CB, 1024], F32, tag="wk")
                nc.scalar.dma_start(
                    wq_f[:], wq_ap[:, :].rearrange("(g p) n -> p g n", p=128)
                )
                nc.scalar.dma_start(
                    wk_f[:], wkv_ap[:, 0:1024].rearrange("(g p) n -> p g n", p=128)
                )

                # q[b, hd] per batch (chained over c blocks, base partition 0)
                q_sb = [
                    consts.tile([1, H * D], F32, tag=f"q{b}", name=f"q{b}")
                    for b in range(BPC)
                ]
                for b in range(BPC):
                    for ch in range(2):
                        q_ps = qps.tile([1, 512], F32, tag="q")
                        for g in range(CB):
                            nc.tensor.matmul(
                                q_ps[:],
                                xcls_f[:, g, b : b + 1],
                                wq_f[:, g, ch * 512 : (ch + 1) * 512],
                                start=(g == 0),
                                stop=(g == CB - 1),
                            )
                        nc.vector.tensor_copy(
                            q_sb[b][:, ch * 512 : (ch + 1) * 512], q_ps[:]
                        )

                # qb[b][c_p, hd] = scale * q[b, hd] broadcast down partitions
                for b in range(BPC):
                    for ch in range(2):
                        qb_ps = qbps.tile([128, 512], F32, tag="qb")
                        nc.tensor.matmul(
                            qb_ps[:],
                            sc_row[0:1, :],
                            q_sb[b][0:1, ch * 512 : (ch + 1) * 512],
                            start=True,
                            stop=True,
                        )
                        nc.vector.tensor_copy(
                            qb_sb[b][:, ch * 512 : (ch + 1) * 512], qb_ps[:]
                        )

                # wkf[b][c, g, h] = scale * sum_d q[b, h*64+d] * Wk[g*128+c, h*64+d]
                for b in range(BPC):
                    for g in range(CB):
                        prod = fold_pool.tile([128, H * D], F32, tag="prod")
                        nc.vector.tensor_mul(prod[:], wk_f[:, g, :], qb_sb[b][:])
                        wkf_g = fold_pool.tile([128, H], F32, tag="wkfg")
                        nc.vector.tensor_reduce(
                            wkf_g[:].unsqueeze(2),
                            prod[:].rearrange("p (h d) -> p h d", d=D),
                            axis=mybir.AxisListType.X,
                            op=mybir.AluOpType.add,
                        )
                        nc.vector.tensor_copy(wkf_bf[b][:, g, :], wkf_g[:])

            # ---- sweeps + finalize ----
            with (
                tc.tile_pool(name="lgps", bufs=2, space="PSUM") as lgps,
                tc.tile_pool(name="xatps", bufs=2, space="PSUM") as xatps,
                tc.tile_pool(name="t16ps", bufs=2, space="PSUM") as t16ps,
                tc.tile_pool(name="esb", bufs=4) as esb_pool,
                tc.tile_pool(name="ebf", bufs=20) as ebf_pool,
                tc.tile_pool(name="small", bufs=2) as small,
            ):
                for b in range(BPC):
                    _emit_batch(
                        nc, b, x_tiles, xt_pool, wkf_bf[b], id16_bf, id16_f,
                        wv_bf, wp_bf, bproj_sb, lgps, xatps, t16ps,
                        esb_pool, ebf_pool, small, out_ap,
                    )


def _emit_batch(nc, b, x_tiles, xt_pool, wkf, id16_bf, id16_f, wv_bf, wp_bf,
                bproj_sb, lgps, xatps, t16ps, esb_pool, ebf_pool, small, out_ap):
    """One batch: 4 chunks of 1024 rows, then finalize.

    Per chunk: XBAR-transpose the whole chunk (one SBUF->SBUF DMA), two
    logits groups of 512 t-cols (8 chained N=512 matmuls each), exp on ACT,
    e transposed back to [t, h] on the PE (tiny), then 16 N=512 matmuls
    accumulate r[h, c] over the whole batch in two PSUM banks.

    PE queue is in-order: chunk k's logits are emitted before chunk k-1's
    eT/r-accumulate so the PE never stalls on the ACT exp dependency."""
    r_psA = xatps.tile([16, 512], F32, tag="xat")
    r_psB = xatps.tile([16, 512], F32, tag="xat")
    sums = small.tile([16, 1], F32, tag=f"sums{b}")

    def emit_xt(k):
        xt = xt_pool.tile([128, SUB * CB, 128], BF16, tag="xt")
        nc.sync.dma_start(xt[:], x_tiles[(b, k)][:], transpose=True)
        return xt

    def emit_logits(k, xt):
        """-> list of e_sb tiles (one per 512-col group)."""
        es = []
        for grp in range(2):
            lg = lgps.tile([16, 512], F32, tag="lg")
            m0 = grp * 4 * CB
            for g in range(CB):
                nc.tensor.matmul(
                    lg[:],
                    wkf[:, g, :],
                    xt[:, m0 + g : m0 + g + 3 * CB + 1 : CB, :],
                    start=(g == 0),
                    stop=(g == CB - 1),
                )
            e_sb = esb_pool.tile([16, 512], BF16, tag="e")
            nc.scalar.activation(e_sb[:], lg[:], mybir.ActivationFunctionType.Exp)
            # running sum_t exp per head, on the DVE
            stmp = small.tile([16, 1], F32, tag="stmp")
            nc.vector.tensor_reduce(
                stmp[:], e_sb[:], axis=mybir.AxisListType.X, op=mybir.AluOpType.add
            )
            if k == 0 and grp == 0:
                nc.vector.tensor_copy(sums[:], stmp[:])
            else:
                nc.vector.tensor_add(sums[:], sums[:], stmp[:])
            es.append(e_sb)
        return es

    def emit_racc(k, es):
        x_bf = x_tiles[(b, k)]
        for grp in range(2):
            e_sb = es[grp]
            for j in range(4):
                i = grp * 4 + j
                eT = t16ps.tile([128, H], BF16, tag="e16")
                nc.tensor.transpose(
                    eT[:], e_sb[:, j * 128 : (j + 1) * 128], id16_bf[:]
                )
                e_bf = ebf_pool.tile([128, H], BF16, tag="ebf")
                nc.vector.tensor_copy(e_bf[:], eT[:])
                first = k == 0 and i == 0
                last = k == NCH - 1 and i == SUB - 1
                nc.tensor.matmul(
                    r_psA[:], e_bf[:], x_bf[:, i, 0:512], start=first, stop=last
                )
                nc.tensor.matmul(
                    r_psB[:], e_bf[:], x_bf[:, i, 512:1024], start=first, stop=last
                )

    # software pipeline, depth 1: logits(k) ahead of racc(k-1)
    pend = None
    xt = emit_xt(0)
    for k in range(NCH):
        cur_xt = xt
        es = emit_logits(k, cur_xt)
        if k + 1 < NCH:
            xt = emit_xt(k + 1)
        if pend is not None:
            emit_racc(*pend)
        pend = (k, es)
    emit_racc(*pend)

    # ---- finalize ----
    rec = small.tile([16, 1], F32, tag="rec")
    nc.vector.reciprocal(rec[:], sums[:])
    r_sb = small.tile([16, C], F32, tag="rsb")
    nc.vector.tensor_scalar_mul(r_sb[:, 0:512], r_psA[:], rec[:])
    nc.vector.tensor_scalar_mul(r_sb[:, 512:1024], r_psB[:], rec[:])

    # rT[c, g, h] = r_sb[h, g*128+c] (PE transpose, tiny)
    rT_bf = small.tile([128, CB, H], BF16, tag="rT")
    for g in range(CB):
        rT_ps = t16ps.tile([128, H], F32, tag="t16")
        nc.tensor.transpose(
            rT_ps[:], r_sb[:, g * 128 : (g + 1) * 128], id16_f[:]
        )
        nc.vector.tensor_copy(rT_bf[:, g, :], rT_ps[:])

    # cls candidates: cls_sb[h, hd] = sum_c (r[h, c]/S[h]) * Wv[c, hd]
    cls_sb = small.tile([16, C], F32, tag="cls")
    for ch in range(2):
        cls_ps = lgps.tile([16, 512], F32, tag="lg")
        for g in range(CB):
            nc.tensor.matmul(
                cls_ps[:],
                rT_bf[:, g, :],
                wv_bf[:, g, ch * 512 : (ch + 1) * 512],
                start=(g == 0),
                stop=(g == CB - 1),
            )
        nc.vector.tensor_copy(cls_sb[:, ch * 512 : (ch + 1) * 512], cls_ps[:])

    # diagonal pick: clsv[hd] = cls_sb[hd//64, hd]
    clsv_bf = small.tile([128, CB], BF16, tag="clsv")
    for g in range(CB):
        aT = t16ps.tile([128, H], F32, tag="t16")
        nc.tensor.transpose(
            aT[:], cls_sb[:, g * 128 : (g + 1) * 128], id16_f[:]
        )
        for half in range(2):
            rows = slice(64 * half, 64 * half + 64)
            nc.vector.tensor_copy(
                clsv_bf[rows, g : g + 1], aT[rows, 2 * g + half : 2 * g + half + 1]
            )

    # out = clsv @ Wproj + bproj
    o_sb = small.tile([1, C], F32, tag="osb")
    for ch in range(2):
        o_ps = lgps.tile([16, 512], F32, tag="lg")
        for g in range(CB):
            nc.tensor.matmul(
                o_ps[0:1, :],
                clsv_bf[:, g : g + 1],
                wp_bf[:, g, ch * 512 : (ch + 1) * 512],
                start=(g == 0),
                stop=(g == CB - 1),
            )
        nc.vector.tensor_add(
            o_sb[0:1, ch * 512 : (ch + 1) * 512],
            o_ps[0:1, :],
            bproj_sb[0:1, ch * 512 : (ch + 1) * 512],
        )
    nc.sync.dma_start(out_ap[b : b + 1, :], o_sb[:])


_CACHED = None


def _get_program():
    global _CACHED
    if _CACHED is None:
        _CACHED = _build()
    return _CACHED


def kernel(x, Wq, Wkv, Wproj, bproj, _trace=False):
    x = np.ascontiguousarray(np.asarray(x, dtype=np.float32))
    Wq = np.ascontiguousarray(np.asarray(Wq, dtype=np.float32))
    Wkv = np.ascontiguousarray(np.asarray(Wkv, dtype=np.float32))
    Wproj = np.ascontiguousarray(np.asarray(Wproj, dtype=np.float32))
    bproj = np.ascontiguousarray(np.asarray(bproj, dtype=np.float32))

    nc = _get_program()
    in_maps = [
        {
            "x": x[cid * BPC : (cid + 1) * BPC],
            "Wq": Wq,
            "Wkv": Wkv,
            "Wproj": Wproj,
            "bproj": bproj,
        }
        for cid in range(N_CORES)
    ]
    res = run_bass_kernel_spmd(
        nc, in_maps, core_ids=list(range(N_CORES)), trace=_trace
    )
    out = np.concatenate([res.results[cid]["out"] for cid in range(N_CORES)], axis=0)
    if _trace:
        kernel.last_exec_time_ns = res.exec_time_ns
        kernel.last_results = res
    return out.reshape(B, 1, C)
